# revision 29
# baseline (speedup 1.0000x reference)
"""Trainium2 Bass kernel for nn_E2EGuidedFilter (guided filter, r=8, eps=0.01).

Full inputs x, y: (8, 3, 1024, 1024) fp32. Data-parallel: one image per
NeuronCore (8 cores).

Per-core pipeline (per channel, H=W=1024, 8 partition-blocks of 128), v12:
  - host-prepped f16 inputs: xcb/ycb = transpose(x-0.5)/transpose(y) in
    layout B (partition = W); xyp = (x-0.5)*y and xx = (x-0.5)^2 in
    layout A; xcbq = transpose(x-0.5)*qw (output normalization
    prefolded). Output f16 layout B; host transposes/casts back.
  - stage-1 x/y fields: H-direction box-sum via tensor_tensor_scan on
    GpSimd over the layout-B free axis (zero-padded buffers), then a
    banded W-matmul (qw folded) back to layout A; the missing qh lands
    in the pointwise ACT evac scales. No PSUM evacuation for these.
  - stage-1 xy/xx fields: H-matmul (qh folded) -> ACT/DVE f16 evac ->
    W-matmul (qw folded), fully normalized.
  - pointwise split per the cost model: ACT consumes PSUM (mx/my/u with
    scale/bias folding: qh, +0.5, +eps), DVE does f16 2x math, GpSimd
    does the bv scalar_tensor_tensor and all scans.
  - stage 2: W-scan (GpSimd) + H-matmul (qh folded) -> layout B;
    final out = z2a*qw*xcb + z2b*qw via ACT-scaled evacs + DVE.
  - num = mean(xyp) - mx*mean(y) = cov(x,y) exactly; the +0.5 output
    shift rides in my' = mean(y) (bv = my' - a*mx).
"""

import os
import sys

import numpy as np

for _p in ("/opt/trn_rl_repo", "/root/.axon_site/_ro/trn_rl_repo"):
    if os.path.isdir(_p) and _p not in sys.path:
        sys.path.append(_p)

R = 8
EPS = 0.01
H = W = 1024
PB = H // 128  # 8 partition blocks
C = 3
NCORES = 8
SCAN_LEN = W + R  # 1032
PAD0 = 18  # interior offset in scan input buffers
CH_PAD = PAD0 + W + 14  # 1056: padded chunk stride
CH_S = SCAN_LEN  # 1032: scan-output / mid chunk stride

_CACHE = {}


def _counts():
    i = np.arange(H)
    return (np.minimum(i + R, H - 1) - np.maximum(i - R, 0) + 1).astype(np.float64)


def _host_consts():
    qh = (1.0 / _counts()).astype(np.float32)

    def band_block(c, lo, n):
        Wt = np.zeros((128, n), np.float32)
        for j in range(n):
            hp = lo + j
            k0 = max(0, hp - R - 128 * c)
            k1 = min(127, hp + R - 128 * c)
            if k0 <= k1:
                Wt[k0 : k1 + 1, j] = qh[hp]
        return Wt

    W0 = band_block(0, 0, 136)
    Wi = band_block(1, 120, 144)
    W7 = band_block(7, 888, 136)
    wq = np.concatenate([W0, Wi, W7], axis=1).astype(np.float16)  # [128,416]
    qv = qh.reshape(PB, 128).T.copy().astype(np.float32)  # [128,8]
    return wq, qv


def _mm_windows():
    halves = [[], []]
    for c in range(PB):
        lo = max(0, 128 * c - 8)
        hi = min(1024, 128 * c + 136)
        if c == 0:
            wt, wbase = "e0", 0
        elif c == PB - 1:
            wt, wbase = "e7", 888
        else:
            wt, wbase = "int", 128 * c - 8
        for hf in (0, 1):
            blo, bhi = 512 * hf, 512 * hf + 512
            s, e = max(lo, blo), min(hi, bhi)
            if s < e:
                halves[hf].append((c, s, e, wt, s - wbase, e - wbase))
    return halves


_HALVES = _mm_windows()


def _split_multi_waits(nc, mybir):
    """This container's walrus supports 1 sync wait per instruction (2 for
    EventSemaphore); Tile emits more. Move excess waits onto NoOps inserted
    just before the instruction on the same engine."""
    uid = [0]
    for f in nc.m.functions:
        for bb in f.blocks:
            out = []
            changed = False
            for inst in bb.instructions:
                si = inst.sync_info
                waits = list(si.on_wait) if si and si.on_wait else []
                cap = 2 if type(inst).__name__ == "InstEventSemaphore" else 1
                if len(waits) > cap:
                    for w in waits[:-cap]:
                        uid[0] += 1
                        nop = mybir.InstNoOp(name=f"wsplit-{uid[0]}", ins=[], outs=[])
                        nop.engine = inst.engine
                        nop.sync_info = mybir.SyncInfo(on_wait=[w], on_update=[])
                        out.append(nop)
                    si.on_wait = waits[-cap:]
                    changed = True
                out.append(inst)
            if changed:
                bb.instructions = out


def _build_bass():
    import concourse.bass as bass
    import concourse.mybir as mybir
    from concourse import tile
    from contextlib import ExitStack

    f16 = mybir.dt.float16
    f32 = mybir.dt.float32
    AF = mybir.ActivationFunctionType
    OP = mybir.AluOpType

    nc = bass.Bass("TRN2", target_bir_lowering=False, debug=False)

    xcb_d = nc.dram_tensor("xcb", [C, PB, 128, W], f16, kind="ExternalInput").ap()
    ycb_d = nc.dram_tensor("ycb", [C, PB, 128, W], f16, kind="ExternalInput").ap()
    xy_d = nc.dram_tensor("xyp", [C, PB, 128, W], f16, kind="ExternalInput").ap()
    xx_d = nc.dram_tensor("xx", [C, PB, 128, W], f16, kind="ExternalInput").ap()
    xcbq_d = nc.dram_tensor("xcbq", [C, PB, 128, W], f16, kind="ExternalInput").ap()
    wq_d = nc.dram_tensor("wq", [128, 416], f16, kind="ExternalInput").ap()
    qv_d = nc.dram_tensor("qv", [128, PB], f32, kind="ExternalInput").ap()
    out_d = nc.dram_tensor("out", [C, PB, 128, W], f16, kind="ExternalOutput").ap()

    with tile.TileContext(nc) as tc, ExitStack() as ctx:
        pconst = ctx.enter_context(tc.tile_pool(name="const", bufs=1))
        wq_t = pconst.tile([128, 416], f16, tag="wq")
        nc.sync.dma_start(wq_t[:], wq_d[:])
        qv_t = pconst.tile([128, PB], f32, tag="qv")
        nc.sync.dma_start(qv_t[:], qv_d[:])

        def wslice(wt, a, b):
            if wt == "e0":
                return wq_t[:, a:b]
            if wt == "int":
                return wq_t[:, 136 + a : 136 + b]
            return wq_t[:, 280 + a : 280 + b]

        # ---- pools ----
        pbig = ctx.enter_context(tc.tile_pool(name="big", bufs=8))
        ppad = ctx.enter_context(tc.tile_pool(name="pad", bufs=1))
        prw = ctx.enter_context(tc.tile_pool(name="ring", bufs=2))
        prw1 = ctx.enter_context(tc.tile_pool(name="ring1", bufs=1))
        pxq = ctx.enter_context(tc.tile_pool(name="xq", bufs=2))
        pout = ctx.enter_context(tc.tile_pool(name="outst", bufs=1))
        pz_h = ctx.enter_context(tc.tile_pool(name="zh", bufs=2, space="PSUM"))
        pz_w = ctx.enter_context(tc.tile_pool(name="zw", bufs=2, space="PSUM"))

        def mm_group_full(z, lhs_of):
            mms = []
            for hf in (0, 1):
                first_in_bank = True
                for c, s, e, wt, wa, wb in _HALVES[hf]:
                    mms.append(
                        (z[:, s:e], lhs_of(c), wslice(wt, wa, wb), first_in_bank)
                    )
                    first_in_bank = False
            for i, (o, l, r, st) in enumerate(mms):
                nc.tensor.matmul(
                    o, l, r,
                    start=st,
                    stop=(i == len(mms) - 1),
                    skip_group_check=True,
                )
            return z

        # padded scan-input buffers for a/b, allocated once; pads memset once
        av_pad = ppad.tile([128, PB * CH_PAD], f16, tag="av_pad")
        bv_pad = ppad.tile([128, PB * CH_PAD], f16, tag="bv_pad")
        for buf in (av_pad, bv_pad):
            for c in range(PB):
                base = c * CH_PAD
                nc.gpsimd.memset(buf[:, base : base + PAD0], 0.0)
                nc.gpsimd.memset(buf[:, base + PAD0 + W : base + CH_PAD], 0.0)

        def memset_pads(buf):
            for c in range(PB):
                base = c * CH_PAD
                nc.gpsimd.memset(buf[:, base : base + PAD0], 0.0)
                nc.gpsimd.memset(buf[:, base + PAD0 + W : base + CH_PAD], 0.0)

        def hscan(dst_big, pad, c, eng=None):
            base = c * CH_PAD
            (eng or nc.gpsimd).tensor_tensor_scan(
                dst_big[:, c * CH_S : (c + 1) * CH_S],
                pad[:, base + PAD0 : base + PAD0 + SCAN_LEN],
                pad[:, base + 1 : base + 1 + SCAN_LEN],
                0.0,
                OP.add,
                OP.subtract,
            )

        def emit_ph2(pch, sA, sB):
            for m in range(PB):
                z2a = pz_w.tile([128, W], f32, tag="zw")
                mm_group_full(
                    z2a,
                    lambda c, _m=m, _s=sA: _s[:, c * CH_S + 8 + 128 * _m : c * CH_S + 8 + 128 * _m + 128],
                )
                z2b = pz_w.tile([128, W], f32, tag="zw")
                mm_group_full(
                    z2b,
                    lambda c, _m=m, _s=sB: _s[:, c * CH_S + 8 + 128 * _m : c * CH_S + 8 + 128 * _m + 128],
                )
                xq = pxq.tile([128, W], f16, tag="xq")
                nc.sync.dma_start(xq[:], xcbq_d[pch, m])
                s2a = prw1.tile([128, W], f16, tag="f1")
                nc.scalar.activation(s2a[:], z2a[:], AF.Copy)
                s2b = prw1.tile([128, W], f16, tag="s2b")
                nc.scalar.activation(
                    s2b[:], z2b[:], AF.Copy, scale=qv_t[:, m : m + 1]
                )
                nc.vector.tensor_mul(s2a[:], s2a[:], xq[:])
                ot = pout.tile([128, W], f16, tag="outst")
                nc.vector.tensor_add(ot[:], s2a[:], s2b[:])
                nc.sync.dma_start(out_d[pch, m], ot[:])

        _bt = [0]

        def big_tile():
            _bt[0] += 1
            return pbig.tile(
                [128, PB * CH_PAD], f16, tag="big", name=f"big{_bt[0]}"
            )

        def load_full(dst_ap, src_t, ch):
            # one DMA for the whole [PB,128,W] channel tensor
            nc.sync.dma_start(dst_ap, src_t[ch])

        prev_ph2 = None
        for ch in range(C):
            # ---- stage 0: loads (batched per tensor) ----
            xcb_pad = big_tile()
            ycb_pad = big_tile()
            xy_big = big_tile()
            xx_big = big_tile()
            if ch == 0:
                memset_pads(xcb_pad)
                memset_pads(ycb_pad)
            load_full(
                xcb_pad[:].rearrange("p (b q) -> b p q", q=CH_PAD)[:, :, PAD0 : PAD0 + W],
                xcb_d, ch,
            )
            load_full(
                ycb_pad[:].rearrange("p (b q) -> b p q", q=CH_PAD)[:, :, PAD0 : PAD0 + W],
                ycb_d, ch,
            )
            load_full(
                xy_big[:, : PB * W].rearrange("p (b q) -> b p q", q=W), xy_d, ch
            )
            load_full(
                xx_big[:, : PB * W].rearrange("p (b q) -> b p q", q=W), xx_d, ch
            )
            if prev_ph2 is not None:
                emit_ph2(prev_ph2[0], prev_ph2[1], prev_ph2[2])
                prev_ph2 = None

            # ---- stage 1a: x/y via H-scan (GpSimd); xy/xx via H-matmul ----
            sx = big_tile()
            for b in range(PB):
                hscan(sx, xcb_pad, b)
            sy = big_tile()
            for b in range(PB):
                hscan(sy, ycb_pad, b, eng=nc.vector)
            mids = {}
            for t, big, ev in (("xy", xy_big, "A"), ("xx", xx_big, "D")):
                midt = big_tile()
                for m in range(PB):
                    zh = pz_h.tile([128, W], f32, tag="zh")
                    mm_group_full(
                        zh,
                        lambda c, _b=big, _m=m: _b[:, c * W + 128 * _m : c * W + 128 * _m + 128],
                    )
                    dst = midt[:, m * CH_S : m * CH_S + W]
                    if ev == "A":
                        nc.scalar.activation(dst, zh[:], AF.Copy)
                    else:
                        nc.vector.tensor_copy(dst, zh[:])
                mids[t] = midt

            # ---- stage 1b: W-matmul (B->A, qw folded) + pointwise ----
            sA = big_tile()
            sB = big_tile()
            for hc in range(PB):
                def wmm_s(sbig, _hc=hc):
                    z = pz_w.tile([128, W], f32, tag="zw")
                    mm_group_full(
                        z,
                        lambda m, _s=sbig: _s[:, m * CH_S + 8 + 128 * _hc : m * CH_S + 8 + 128 * _hc + 128],
                    )
                    return z

                def wmm_m(t, _hc=hc):
                    z = pz_w.tile([128, W], f32, tag="zw")
                    mm_group_full(
                        z,
                        lambda m, _t=t: mids[_t][:, m * CH_S + 128 * _hc : m * CH_S + 128 * _hc + 128],
                    )
                    return z

                qs = qv_t[:, hc : hc + 1]
                z_x = wmm_s(sx)
                mx = prw.tile([128, W], f16, tag="mx")
                nc.scalar.activation(mx[:], z_x[:], AF.Copy, scale=qs)
                z_y = wmm_s(sy)
                my = prw.tile([128, W], f16, tag="my")
                nc.scalar.activation(my[:], z_y[:], AF.Copy, scale=qs)
                t1 = prw.tile([128, W], f16, tag="t1")
                nc.vector.tensor_mul(t1[:], mx[:], my[:])
                z_xy = wmm_m("xy")
                num = prw.tile([128, W], f16, tag="num")
                nc.vector.tensor_sub(num[:], z_xy[:], t1[:])
                z_xx = wmm_m("xx")
                u = prw.tile([128, W], f16, tag="u")
                nc.scalar.activation(u[:], z_xx[:], AF.Copy, bias=EPS)
                s2 = prw.tile([128, W], f16, tag="s2")
                nc.scalar.activation(s2[:], mx[:], AF.Square)
                nc.vector.tensor_sub(u[:], u[:], s2[:])  # u <- den
                with nc.allow_low_precision(
                    reason="18-bit reciprocal ample for eps-regularized den"
                ):
                    nc.vector.reciprocal(u[:], u[:])  # u <- 1/den
                avc = av_pad[:, hc * CH_PAD + PAD0 : hc * CH_PAD + PAD0 + W]
                nc.vector.tensor_mul(avc, num[:], u[:])
                nc.vector.tensor_mul(mx[:], avc, mx[:])  # mx <- j1 = a*mx
                # bv = my' - j1  (my' = mean(y); 0.5 shift folded in)
                bvc = bv_pad[:, hc * CH_PAD + PAD0 : hc * CH_PAD + PAD0 + W]
                nc.gpsimd.scalar_tensor_tensor(
                    bvc, my[:], 0.0, mx[:], OP.add, OP.subtract
                )
                hscan(sA, av_pad, hc)
                hscan(sB, bv_pad, hc)

            prev_ph2 = (ch, sA, sB)
        emit_ph2(prev_ph2[0], prev_ph2[1], prev_ph2[2])

    _split_multi_waits(nc, mybir)
    return nc


def _get_bass():
    if "nc" not in _CACHE:
        _CACHE["nc"] = _build_bass()
    return _CACHE["nc"]


def kernel(x, y):
    x = np.asarray(x)
    y = np.asarray(y)
    from concourse.bass_utils import run_bass_kernel_spmd

    nc = _get_bass()
    wq, qv = _host_consts()
    B = x.shape[0]
    xcf = (x - 0.5).astype(np.float16)
    xcbf = np.ascontiguousarray(xcf.transpose(0, 1, 3, 2))
    ycbf = np.ascontiguousarray(y.transpose(0, 1, 3, 2).astype(np.float16))
    xypf = (xcf.astype(np.float32) * y).astype(np.float16)
    xxf = (xcf.astype(np.float32) ** 2).astype(np.float16)
    qw_w = (1.0 / _counts()).astype(np.float32)
    xcbqf = (xcbf.astype(np.float32) * qw_w[None, None, :, None]).astype(np.float16)
    sh = (B, C, PB, 128, W)
    in_maps = [
        {
            "xcb": xcbf.reshape(sh)[i],
            "ycb": ycbf.reshape(sh)[i],
            "xyp": xypf.reshape(sh)[i],
            "xx": xxf.reshape(sh)[i],
            "xcbq": xcbqf.reshape(sh)[i],
            "wq": wq,
            "qv": qv,
        }
        for i in range(B)
    ]
    res = run_bass_kernel_spmd(nc, in_maps, core_ids=list(range(B)))
    out = np.stack(
        [
            res.results[i]["out"].reshape(C, W, H).transpose(0, 2, 1)
            for i in range(B)
        ]
    )
    return np.ascontiguousarray(out).astype(np.float32)
